# revision 3
# baseline (speedup 1.0000x reference)
"""Single-head causal attention (B=8, T=2048, C=1024, H=64) on 8 TRN2 NeuronCores.

Data-parallel over batch: core b computes attention for batch element b.

Device algorithm (per core), all fp32:
  - Inputs pre-marshalled on host: aT = a.T  [C=1024, T=2048], Wqv = [Wq*scale | Wv]
    [1024, 128], Wk [1024, 64].
  - Projections: qT/vT from lhsT=Wqv tiles, kT from lhsT=Wk tiles, rhs = aT
    C-tiles; outputs land as qT [64, T] (partitions 0-63), vT (partitions
    64-127), kT [64, T].
  - v natural [T-tile, 65] built by PE transpose of vT 128-col chunks with an
    identity moving operand; column 64 is set to 1.0 (ones column).
  - Scores computed transposed: sT[tk, tq] via lhsT = kT tile [64, 128],
    rhs = qT chunk [64, 512] (contraction H=64).  exp on ScalarE directly from
    PSUM in [128, 1024] groups (2 k-tiles per op).  Causal mask = elementwise
    multiply with precomputed 0/1 masks on the 4 diagonal k-tiles per chunk.
  - PV: out_T/denom accumulate in one PSUM group: lhsT = [v | 1] [128, 65],
    rhs = expT group slices; row 64 of the [65, 512] accumulator is the
    softmax denominator (sum of exps).  No max-subtraction is needed: logits
    are ~N(0, ~1.5), max < ~10, exp is safely in fp32 range.
  - Normalize: reciprocal of denom row, broadcast across 64 partitions with a
    K=1 ones matmul, multiply, DMA out as outT [64, T].  Host transposes back.

T is processed in 4 chunks of 512 q-columns; aT is DMAed in T-quarters so
chunk j's entire dependency set (q, k, v cols <= 512(j+1)) arrives early and
compute overlaps the HBM stream.
"""

import sys

sys.path.insert(0, "/opt/trn_rl_repo")
sys.path.insert(0, "/root/.axon_site")

import numpy as np

import concourse.bass as bass
import concourse.mybir as mybir
import concourse.tile as tile
from concourse import bacc
from concourse import bass_utils

B, T, C, H = 8, 2048, 1024, 64
P = 128
NCT = C // P          # 8 C-tiles (contraction)
CHUNK = 512           # q-columns per chunk
NCH = T // CHUNK      # 4 chunks
NKT = T // P          # 16 k-tiles
SCALE = H ** -0.5
FP = mybir.dt.float32

_cache = {}


def build_program():
    nc = bacc.Bacc("TRN2", target_bir_lowering=False, debug=False)

    aT = nc.dram_tensor("aT", [C, T], FP, kind="ExternalInput").ap()
    wqv = nc.dram_tensor("wqv", [C, 2 * H], FP, kind="ExternalInput").ap()
    wk = nc.dram_tensor("wk", [C, H], FP, kind="ExternalInput").ap()
    idh = nc.dram_tensor("idh", [P, H], FP, kind="ExternalInput").ap()
    m4 = nc.dram_tensor("m4", [P, 4 * CHUNK], FP, kind="ExternalInput").ap()
    outT = nc.dram_tensor("outT", [H, T], FP, kind="ExternalOutput").ap()

    with tile.TileContext(nc) as tc:
        with (
            tc.tile_pool(name="const", bufs=1) as const_pool,
            tc.tile_pool(name="at", bufs=NCT * NCH) as at_pool,
            tc.tile_pool(name="qv", bufs=1) as qv_pool,
            tc.tile_pool(name="kt", bufs=1) as kt_pool,
            tc.tile_pool(name="v1", bufs=NKT) as v1_pool,
            tc.tile_pool(name="es", bufs=3) as e_pool,
            tc.tile_pool(name="norm", bufs=4) as norm_pool,
            tc.tile_pool(name="out", bufs=1) as out_pool,
            tc.tile_pool(name="ps_s", bufs=1, space="PSUM") as s_psum,
            tc.tile_pool(name="ps_proj", bufs=2, space="PSUM") as proj_psum,
            tc.tile_pool(name="ps_pv", bufs=2, space="PSUM") as pv_psum,
            tc.tile_pool(name="ps_small", bufs=2, space="PSUM") as small_psum,
        ):
            # ---- constants ----
            wqv_sb = const_pool.tile([P, NCT, 2 * H], FP, tag="wqv")
            nc.sync.dma_start(wqv_sb[:], wqv.rearrange("(ko p) m -> p ko m", p=P))
            wk_sb = const_pool.tile([P, NCT, H], FP, tag="wk")
            nc.sync.dma_start(wk_sb[:], wk.rearrange("(ko p) m -> p ko m", p=P))
            idh_sb = const_pool.tile([P, H], FP, tag="idh")
            nc.sync.dma_start(idh_sb[:], idh[:])
            m4_sb = const_pool.tile([P, 4 * CHUNK], FP, tag="m4")
            nc.sync.dma_start(m4_sb[:], m4[:])
            ones_sb = const_pool.tile([P, H], FP, tag="ones")
            nc.vector.memset(ones_sb[H : H + 1, :], 1.0)

            # ---- aT quarters, streamed in chunk order ----
            at_sb = {}
            for j in range(NCH):
                for c in range(NCT):
                    t_ = at_pool.tile([P, CHUNK], FP, tag="at")
                    nc.sync.dma_start(
                        t_[:],
                        aT[c * P : (c + 1) * P, j * CHUNK : (j + 1) * CHUNK],
                    )
                    at_sb[(c, j)] = t_

            qv_sb = qv_pool.tile([P, T], FP, tag="qv")   # q rows 0-63, vT rows 64-127
            kT_sb = kt_pool.tile([H, T], FP, tag="kt")
            outT_sb = out_pool.tile([H, T], FP, tag="ot")
            v1 = {}

            for j in range(NCH):
                cs = slice(j * CHUNK, (j + 1) * CHUNK)

                # ---- projections for this chunk of T ----
                ps_qv = proj_psum.tile([P, CHUNK], FP, tag="proj")
                for c in range(NCT):
                    nc.tensor.matmul(
                        ps_qv[:], wqv_sb[:, c, :], at_sb[(c, j)][:],
                        start=(c == 0), stop=(c == NCT - 1),
                    )
                ps_k = proj_psum.tile([P, CHUNK], FP, tag="proj")
                for c in range(NCT):
                    nc.tensor.matmul(
                        ps_k[:H], wk_sb[:, c, :], at_sb[(c, j)][:],
                        start=(c == 0), stop=(c == NCT - 1),
                    )
                nc.vector.tensor_copy(qv_sb[:, cs], ps_qv[:])
                nc.vector.tensor_copy(kT_sb[:, cs], ps_k[:H])

                # ---- v natural tiles ([v | 1], PE transpose of vT chunks) ----
                for kt in range(4 * j, 4 * j + 4):
                    ps_t = small_psum.tile([P, H], FP, tag="small")
                    nc.tensor.transpose(
                        ps_t[:],
                        qv_sb[H:P, kt * P : (kt + 1) * P],
                        idh_sb[H:P, :],
                    )
                    vt = v1_pool.tile([P, H + 1], FP, tag="v1")
                    nc.vector.memset(vt[:, H : H + 1], 1.0)
                    nc.vector.tensor_copy(vt[:, :H], ps_t[:])
                    v1[kt] = vt

                # ---- attention: groups of 2 k-tiles ----
                ps_o = pv_psum.tile([H + 1, CHUNK], FP, tag="pv")
                nkt_j = 4 * j + 4          # k-tiles for this chunk (causal)
                for g in range(nkt_j // 2):
                    kts = [2 * g, 2 * g + 1]
                    ps_s = s_psum.tile([P, 2 * CHUNK], FP, tag="s")
                    for i, kt in enumerate(kts):
                        nc.tensor.matmul(
                            ps_s[:, i * CHUNK : (i + 1) * CHUNK],
                            kT_sb[:, kt * P : (kt + 1) * P],
                            qv_sb[:H, cs],
                            start=True, stop=True,
                        )
                    e_sb = e_pool.tile([P, 2 * CHUNK], FP, tag="e")
                    nc.scalar.activation(
                        e_sb[:], ps_s[:], mybir.ActivationFunctionType.Exp
                    )
                    r0 = 2 * g - 4 * j      # diagonal offset of first kt in group
                    if r0 >= 0:             # diagonal group: causal mask
                        nc.vector.tensor_mul(
                            e_sb[:], e_sb[:],
                            m4_sb[:, r0 * CHUNK : (r0 + 2) * CHUNK],
                        )
                    for i, kt in enumerate(kts):
                        nc.tensor.matmul(
                            ps_o[:],
                            v1[kt][:],
                            e_sb[:, i * CHUNK : (i + 1) * CHUNK],
                            start=(kt == 0), stop=(kt == nkt_j - 1),
                        )

                # ---- normalize: out[h, tq] * 1/denom[tq] ----
                den = norm_pool.tile([H + 1, CHUNK], FP, tag="den")
                nc.vector.tensor_copy(den[H : H + 1, :], ps_o[H : H + 1, :])
                rec = norm_pool.tile([H + 1, CHUNK], FP, tag="rec")
                nc.vector.reciprocal(rec[H : H + 1, :], den[H : H + 1, :])
                ps_b = small_psum.tile([H, CHUNK], FP, tag="small")
                nc.tensor.matmul(
                    ps_b[:], ones_sb[H : H + 1, :], rec[H : H + 1, :],
                    start=True, stop=True,
                )
                bc_sb = norm_pool.tile([H, CHUNK], FP, tag="bc")
                nc.vector.tensor_copy(bc_sb[:], ps_b[:])
                nc.vector.tensor_mul(outT_sb[:, cs], ps_o[:H, :], bc_sb[:])
                nc.sync.dma_start(outT[:, cs], outT_sb[:, cs])

    nc.compile()
    return nc


def _marshal(a, Wk, Wq, Wv):
    aT = np.ascontiguousarray(a.transpose(0, 2, 1))            # [B, C, T]
    wqv = np.ascontiguousarray(
        np.concatenate([Wq * np.float32(SCALE), Wv], axis=1)
    )                                                          # [C, 128]
    idh = np.zeros((P, H), np.float32)
    idh[H:P, :] = np.eye(H, dtype=np.float32)
    m4 = np.zeros((P, 4 * CHUNK), np.float32)
    p = np.arange(P)[:, None]
    f = np.arange(CHUNK)[None, :]
    for r in range(4):
        m4[:, r * CHUNK : (r + 1) * CHUNK] = (f >= r * P + p).astype(np.float32)
    return aT, wqv, np.ascontiguousarray(Wk), idh, m4


def kernel(a, Wk, Wq, Wv):
    a = np.asarray(a, np.float32)
    Wk = np.asarray(Wk, np.float32)
    Wq = np.asarray(Wq, np.float32)
    Wv = np.asarray(Wv, np.float32)
    if "nc" not in _cache:
        _cache["nc"] = build_program()
    nc = _cache["nc"]

    aT, wqv, wk, idh, m4 = _marshal(a, Wk, Wq, Wv)
    in_maps = [
        {"aT": aT[b], "wqv": wqv, "wk": wk, "idh": idh, "m4": m4}
        for b in range(B)
    ]
    res = bass_utils.run_bass_kernel_spmd(nc, in_maps, core_ids=list(range(B)))
    out = np.stack(
        [np.ascontiguousarray(res.results[b]["outT"].T) for b in range(B)]
    )
    return out.astype(np.float32)


# revision 7
# speedup vs baseline: 1.8876x; 1.8876x over previous
"""Single-head causal attention (B=8, T=2048, C=1024, H=64) on 8 TRN2 NeuronCores.

Data-parallel over batch: core b computes attention for batch element b.

Device algorithm (per core), all fp32:
  - Inputs pre-marshalled on host: aT = a.T  [C=1024, T=2048], Wqv = [Wq*scale | Wv]
    [1024, 128], Wk [1024, 64].
  - Projections: qT/vT from lhsT=Wqv tiles, kT from lhsT=Wk tiles, rhs = aT
    C-tiles; outputs land as qT [64, T] (partitions 0-63), vT (partitions
    64-127), kT [64, T].
  - v natural [T-tile, 65] built by PE transpose of vT 128-col chunks with an
    identity moving operand; column 64 is set to 1.0 (ones column).
  - Scores computed transposed: sT[tk, tq] via lhsT = kT tile [64, 128],
    rhs = qT chunk [64, 512] (contraction H=64).  exp on ScalarE directly from
    PSUM in [128, 1024] groups (2 k-tiles per op).  Causal mask = elementwise
    multiply with precomputed 0/1 masks on the 4 diagonal k-tiles per chunk.
  - PV: out_T/denom accumulate in one PSUM group: lhsT = [v | 1] [128, 65],
    rhs = expT group slices; row 64 of the [65, 512] accumulator is the
    softmax denominator (sum of exps).  No max-subtraction is needed: logits
    are ~N(0, ~1.5), max < ~10, exp is safely in fp32 range.
  - Normalize: reciprocal of denom row, broadcast across 64 partitions with a
    K=1 ones matmul, multiply, DMA out as outT [64, T].  Host transposes back.

T is processed in 4 chunks of 512 q-columns; aT is DMAed in T-quarters so
chunk j's entire dependency set (q, k, v cols <= 512(j+1)) arrives early and
compute overlaps the HBM stream.
"""

import sys

sys.path.insert(0, "/opt/trn_rl_repo")
sys.path.insert(0, "/root/.axon_site")

import numpy as np

import concourse.bass as bass
import concourse.mybir as mybir
import concourse.tile as tile
from concourse import bacc
from concourse import bass_utils

B, T, C, H = 8, 2048, 1024, 64
P = 128
NCT = C // P          # 8 C-tiles (contraction)
CHUNK = 512           # q-columns per chunk
NCH = T // CHUNK      # 4 chunks
NKT = T // P          # 16 k-tiles
SCALE = H ** -0.5
FP = mybir.dt.float32
FPR = mybir.dt.float32r   # 11-bit-mantissa RNE matmul mode, 3x faster than fp32

_cache = {}


def build_program():
    nc = bacc.Bacc("TRN2", target_bir_lowering=False, debug=False)

    aT = nc.dram_tensor("aT", [C, T], FPR, kind="ExternalInput").ap()
    wqv = nc.dram_tensor("wqv", [C, 2 * H], FPR, kind="ExternalInput").ap()
    wk = nc.dram_tensor("wk", [C, H], FPR, kind="ExternalInput").ap()
    idh = nc.dram_tensor("idh", [P, H], FPR, kind="ExternalInput").ap()
    m4 = nc.dram_tensor("m4", [P, 4 * CHUNK], FPR, kind="ExternalInput").ap()
    ones = nc.dram_tensor("ones", [P, H], FPR, kind="ExternalInput").ap()
    outT = nc.dram_tensor("outT", [H, T], FP, kind="ExternalOutput").ap()

    with tile.TileContext(nc) as tc:
        with (
            tc.tile_pool(name="const", bufs=1) as const_pool,
            tc.tile_pool(name="at", bufs=NCT * NCH) as at_pool,
            tc.tile_pool(name="qv", bufs=1) as qv_pool,
            tc.tile_pool(name="kt", bufs=1) as kt_pool,
            tc.tile_pool(name="v1", bufs=NKT) as v1_pool,
            tc.tile_pool(name="es", bufs=3) as e_pool,
            tc.tile_pool(name="norm", bufs=4) as norm_pool,
            tc.tile_pool(name="out", bufs=1) as out_pool,
            tc.tile_pool(name="ps_s", bufs=1, space="PSUM") as s_psum,
            tc.tile_pool(name="ps_proj", bufs=2, space="PSUM") as proj_psum,
            tc.tile_pool(name="ps_pv", bufs=2, space="PSUM") as pv_psum,
            tc.tile_pool(name="ps_small", bufs=2, space="PSUM") as small_psum,
        ):
            # ---- constants ----
            wqv_sb = const_pool.tile([P, NCT, 2 * H], FPR, tag="wqv")
            nc.sync.dma_start(wqv_sb[:], wqv.rearrange("(ko p) m -> p ko m", p=P))
            wk_sb = const_pool.tile([P, NCT, H], FPR, tag="wk")
            nc.sync.dma_start(wk_sb[:], wk.rearrange("(ko p) m -> p ko m", p=P))
            idh_sb = const_pool.tile([P, H], FPR, tag="idh")
            nc.sync.dma_start(idh_sb[:], idh[:])
            m4_sb = const_pool.tile([P, 4 * CHUNK], FPR, tag="m4")
            nc.sync.dma_start(m4_sb[:], m4[:])
            ones_sb = const_pool.tile([P, H], FPR, tag="ones")
            nc.sync.dma_start(ones_sb[:], ones[:])

            # ---- aT quarters, streamed in chunk order ----
            at_sb = {}
            for j in range(NCH):
                for c in range(NCT):
                    t_ = at_pool.tile([P, CHUNK], FPR, tag="at")
                    nc.sync.dma_start(
                        t_[:],
                        aT[c * P : (c + 1) * P, j * CHUNK : (j + 1) * CHUNK],
                    )
                    at_sb[(c, j)] = t_

            qv_sb = qv_pool.tile([P, T], FPR, tag="qv")   # q rows 0-63, vT rows 64-127
            kT_sb = kt_pool.tile([H, T], FPR, tag="kt")
            outT_sb = out_pool.tile([H, T], FP, tag="ot")
            v1 = {}

            for j in range(NCH):
                cs = slice(j * CHUNK, (j + 1) * CHUNK)

                # ---- projections for this chunk of T ----
                ps_qv = proj_psum.tile([P, CHUNK], FP, tag="proj")
                for c in range(NCT):
                    nc.tensor.matmul(
                        ps_qv[:], wqv_sb[:, c, :], at_sb[(c, j)][:],
                        start=(c == 0), stop=(c == NCT - 1),
                    )
                ps_k = proj_psum.tile([P, CHUNK], FP, tag="proj")
                for c in range(NCT):
                    nc.tensor.matmul(
                        ps_k[:H], wk_sb[:, c, :], at_sb[(c, j)][:],
                        start=(c == 0), stop=(c == NCT - 1),
                    )
                nc.vector.tensor_copy(qv_sb[:, cs], ps_qv[:])
                nc.vector.tensor_copy(kT_sb[:, cs], ps_k[:H])

                # ---- v natural tiles ([v | 1], PE transpose of vT chunks) ----
                for kt in range(4 * j, 4 * j + 4):
                    ps_t = small_psum.tile([P, H], FPR, tag="small")
                    nc.tensor.transpose(
                        ps_t[:],
                        qv_sb[H:P, kt * P : (kt + 1) * P],
                        idh_sb[H:P, :],
                    )
                    vt = v1_pool.tile([P, H + 1], FPR, tag="v1")
                    nc.vector.tensor_copy(vt[:, H : H + 1], ones_sb[:, :1])
                    nc.vector.tensor_copy(vt[:, :H], ps_t[:])
                    v1[kt] = vt

                # ---- attention: groups of 2 k-tiles ----
                ps_o = pv_psum.tile([H + 1, CHUNK], FP, tag="pv")
                nkt_j = 4 * j + 4          # k-tiles for this chunk (causal)
                for g in range(nkt_j // 2):
                    kts = [2 * g, 2 * g + 1]
                    ps_s = s_psum.tile([P, 2 * CHUNK], FP, tag="s")
                    for i, kt in enumerate(kts):
                        nc.tensor.matmul(
                            ps_s[:, i * CHUNK : (i + 1) * CHUNK],
                            kT_sb[:, kt * P : (kt + 1) * P],
                            qv_sb[:H, cs],
                            start=True, stop=True,
                        )
                    e_sb = e_pool.tile([P, 2 * CHUNK], FPR, tag="e")
                    nc.scalar.activation(
                        e_sb[:], ps_s[:], mybir.ActivationFunctionType.Exp
                    )
                    r0 = 2 * g - 4 * j      # diagonal offset of first kt in group
                    if r0 >= 0:             # diagonal group: causal mask
                        nc.vector.tensor_mul(
                            e_sb[:], e_sb[:],
                            m4_sb[:, r0 * CHUNK : (r0 + 2) * CHUNK],
                        )
                    for i, kt in enumerate(kts):
                        nc.tensor.matmul(
                            ps_o[:],
                            v1[kt][:],
                            e_sb[:, i * CHUNK : (i + 1) * CHUNK],
                            start=(kt == 0), stop=(kt == nkt_j - 1),
                        )

                # ---- normalize: out[h, tq] * 1/denom[tq] ----
                rec = norm_pool.tile([H + 1, CHUNK], FPR, tag="rec")
                with nc.allow_low_precision(reason="fp32r denom reciprocal"):
                    nc.vector.reciprocal(rec[:], ps_o[:])
                ps_b = small_psum.tile([H, CHUNK], FP, tag="small")
                nc.tensor.matmul(
                    ps_b[:], ones_sb[H : H + 1, :], rec[H : H + 1, :],
                    start=True, stop=True,
                )
                bc_sb = norm_pool.tile([H, CHUNK], FP, tag="bc")
                nc.vector.tensor_copy(bc_sb[:], ps_b[:])
                nc.vector.tensor_mul(outT_sb[:, cs], ps_o[:H, :], bc_sb[:])
                nc.sync.dma_start(outT[:, cs], outT_sb[:, cs])

    nc.compile()
    return nc


def _marshal(a, Wk, Wq, Wv):
    aT = np.ascontiguousarray(a.transpose(0, 2, 1))            # [B, C, T]
    wqv = np.ascontiguousarray(
        np.concatenate([Wq * np.float32(SCALE), Wv], axis=1)
    )                                                          # [C, 128]
    idh = np.zeros((P, H), np.float32)
    idh[H:P, :] = np.eye(H, dtype=np.float32)
    m4 = np.zeros((P, 4 * CHUNK), np.float32)
    p = np.arange(P)[:, None]
    f = np.arange(CHUNK)[None, :]
    for r in range(4):
        m4[:, r * CHUNK : (r + 1) * CHUNK] = (f >= r * P + p).astype(np.float32)
    ones = np.ones((P, H), np.float32)
    return aT, wqv, np.ascontiguousarray(Wk), idh, m4, ones


def kernel(a, Wk, Wq, Wv):
    a = np.asarray(a, np.float32)
    Wk = np.asarray(Wk, np.float32)
    Wq = np.asarray(Wq, np.float32)
    Wv = np.asarray(Wv, np.float32)
    if "nc" not in _cache:
        _cache["nc"] = build_program()
    nc = _cache["nc"]

    aT, wqv, wk, idh, m4, ones = _marshal(a, Wk, Wq, Wv)
    in_maps = [
        {"aT": aT[b], "wqv": wqv, "wk": wk, "idh": idh, "m4": m4, "ones": ones}
        for b in range(B)
    ]
    res = bass_utils.run_bass_kernel_spmd(nc, in_maps, core_ids=list(range(B)))
    out = np.stack(
        [np.ascontiguousarray(res.results[b]["outT"].T) for b in range(B)]
    )
    return out.astype(np.float32)


# revision 9
# speedup vs baseline: 2.2427x; 1.1881x over previous
"""Single-head causal attention (B=8, T=2048, C=1024, H=64) on 8 TRN2 NeuronCores.

Data-parallel over batch: core b computes attention for batch element b.

Device algorithm (per core), all fp32:
  - Inputs pre-marshalled on host: aT = a.T  [C=1024, T=2048], Wqv = [Wq*scale | Wv]
    [1024, 128], Wk [1024, 64].
  - Projections: qT/vT from lhsT=Wqv tiles, kT from lhsT=Wk tiles, rhs = aT
    C-tiles; outputs land as qT [64, T] (partitions 0-63), vT (partitions
    64-127), kT [64, T].
  - v natural [T-tile, 65] built by PE transpose of vT 128-col chunks with an
    identity moving operand; column 64 is set to 1.0 (ones column).
  - Scores computed transposed: sT[tk, tq] via lhsT = kT tile [64, 128],
    rhs = qT chunk [64, 512] (contraction H=64).  exp on ScalarE directly from
    PSUM in [128, 1024] groups (2 k-tiles per op).  Causal mask = elementwise
    multiply with precomputed 0/1 masks on the 4 diagonal k-tiles per chunk.
  - PV: out_T/denom accumulate in one PSUM group: lhsT = [v | 1] [128, 65],
    rhs = expT group slices; row 64 of the [65, 512] accumulator is the
    softmax denominator (sum of exps).  No max-subtraction is needed: logits
    are ~N(0, ~1.5), max < ~10, exp is safely in fp32 range.
  - Normalize: reciprocal of denom row, broadcast across 64 partitions with a
    K=1 ones matmul, multiply, DMA out as outT [64, T].  Host transposes back.

T is processed in 4 chunks of 512 q-columns; aT is DMAed in T-quarters so
chunk j's entire dependency set (q, k, v cols <= 512(j+1)) arrives early and
compute overlaps the HBM stream.
"""

import sys

sys.path.insert(0, "/opt/trn_rl_repo")
sys.path.insert(0, "/root/.axon_site")

import numpy as np

import concourse.bass as bass
import concourse.mybir as mybir
import concourse.tile as tile
from concourse import bacc
from concourse import bass_utils

B, T, C, H = 8, 2048, 1024, 64
P = 128
NCT = C // P          # 8 C-tiles (contraction)
CHUNK = 512           # q-columns per chunk
NCH = T // CHUNK      # 4 chunks
NKT = T // P          # 16 k-tiles
SCALE = H ** -0.5
FP = mybir.dt.float32
FPR = mybir.dt.float32r   # 11-bit-mantissa RNE matmul mode, 3x faster than fp32

_cache = {}


def build_program():
    nc = bacc.Bacc("TRN2", target_bir_lowering=False, debug=False)

    aT = nc.dram_tensor("aT", [C, T], FPR, kind="ExternalInput").ap()
    wqv = nc.dram_tensor("wqv", [C, 2 * H], FPR, kind="ExternalInput").ap()
    wk = nc.dram_tensor("wk", [C, H], FPR, kind="ExternalInput").ap()
    idh = nc.dram_tensor("idh", [P, H], FPR, kind="ExternalInput").ap()
    m4 = nc.dram_tensor("m4", [P, 4 * CHUNK], FPR, kind="ExternalInput").ap()
    ones = nc.dram_tensor("ones", [P, H], FPR, kind="ExternalInput").ap()
    outT = nc.dram_tensor("outT", [H, T], FP, kind="ExternalOutput").ap()

    with tile.TileContext(nc) as tc:
        with (
            tc.tile_pool(name="const", bufs=1) as const_pool,
            tc.tile_pool(name="at", bufs=NCT * NCH) as at_pool,
            tc.tile_pool(name="qv", bufs=1) as qv_pool,
            tc.tile_pool(name="kt", bufs=1) as kt_pool,
            tc.tile_pool(name="v1", bufs=NKT) as v1_pool,
            tc.tile_pool(name="es", bufs=3) as e_pool,
            tc.tile_pool(name="norm", bufs=4) as norm_pool,
            tc.tile_pool(name="out", bufs=1) as out_pool,
            tc.tile_pool(name="ps_s", bufs=2, space="PSUM") as s_psum,
            tc.tile_pool(name="ps_proj", bufs=2, space="PSUM") as proj_psum,
            tc.tile_pool(name="ps_pv", bufs=1, space="PSUM") as pv_psum,
            tc.tile_pool(name="ps_small", bufs=1, space="PSUM") as small_psum,
        ):
            # ---- constants ----
            wqv_sb = const_pool.tile([P, NCT, 2 * H], FPR, tag="wqv")
            nc.sync.dma_start(wqv_sb[:], wqv.rearrange("(ko p) m -> p ko m", p=P))
            wk_sb = const_pool.tile([P, NCT, H], FPR, tag="wk")
            nc.sync.dma_start(wk_sb[:], wk.rearrange("(ko p) m -> p ko m", p=P))
            # ---- aT quarters, streamed in chunk order ----
            at_sb = {}
            for j in range(NCH):
                for c in range(NCT):
                    t_ = at_pool.tile([P, CHUNK], FPR, tag="at")
                    eng = nc.sync if j == 0 else nc.gpsimd
                    eng.dma_start(
                        t_[:],
                        aT[c * P : (c + 1) * P, j * CHUNK : (j + 1) * CHUNK],
                    )
                    at_sb[(c, j)] = t_

            idh_sb = const_pool.tile([P, H], FPR, tag="idh")
            nc.sync.dma_start(idh_sb[:], idh[:])
            m4_sb = const_pool.tile([P, 4 * CHUNK], FPR, tag="m4")
            nc.sync.dma_start(m4_sb[:], m4[:])
            ones_sb = const_pool.tile([P, H], FPR, tag="ones")
            nc.sync.dma_start(ones_sb[:], ones[:])

            qv_sb = qv_pool.tile([P, T], FPR, tag="qv")   # q rows 0-63, vT rows 64-127
            kT_sb = kt_pool.tile([H, T], FPR, tag="kt")
            outT_sb = out_pool.tile([H, T], FP, tag="ot")
            v1 = {}

            for j in range(NCH):
                cs = slice(j * CHUNK, (j + 1) * CHUNK)

                # ---- projections for this chunk of T ----
                ps_qv = proj_psum.tile([P, CHUNK], FP, tag="proj")
                for c in range(NCT):
                    nc.tensor.matmul(
                        ps_qv[:], wqv_sb[:, c, :], at_sb[(c, j)][:],
                        start=(c == 0), stop=(c == NCT - 1),
                    )
                ps_k = proj_psum.tile([P, CHUNK], FP, tag="proj")
                for c in range(NCT):
                    nc.tensor.matmul(
                        ps_k[:H], wk_sb[:, c, :], at_sb[(c, j)][:],
                        start=(c == 0), stop=(c == NCT - 1),
                    )
                nc.vector.tensor_copy(qv_sb[:, cs], ps_qv[:])
                nc.vector.tensor_copy(kT_sb[:, cs], ps_k[:H])

                # ---- v natural tiles ([v | 1], PE transpose of vT chunks) ----
                for kt in range(4 * j, 4 * j + 4):
                    ps_t = small_psum.tile([P, H], FPR, tag="small")
                    nc.tensor.transpose(
                        ps_t[:],
                        qv_sb[H:P, kt * P : (kt + 1) * P],
                        idh_sb[H:P, :],
                    )
                    vt = v1_pool.tile([P, H + 1], FPR, tag="v1")
                    nc.vector.tensor_copy(vt[:, H : H + 1], ones_sb[:, :1])
                    nc.vector.tensor_copy(vt[:, :H], ps_t[:])
                    v1[kt] = vt

                # ---- attention: groups of 2 k-tiles ----
                ps_o = pv_psum.tile([H + 1, CHUNK], FP, tag="pv")
                nkt_j = 4 * j + 4          # k-tiles for this chunk (causal)
                for g in range(nkt_j // 2):
                    kts = [2 * g, 2 * g + 1]
                    ps_s = s_psum.tile([P, 2 * CHUNK], FP, tag="s")
                    for i, kt in enumerate(kts):
                        nc.tensor.matmul(
                            ps_s[:, i * CHUNK : (i + 1) * CHUNK],
                            kT_sb[:, kt * P : (kt + 1) * P],
                            qv_sb[:H, cs],
                            start=True, stop=True,
                        )
                    e_sb = e_pool.tile([P, 2 * CHUNK], FPR, tag="e")
                    nc.scalar.activation(
                        e_sb[:], ps_s[:], mybir.ActivationFunctionType.Exp
                    )
                    r0 = 2 * g - 4 * j      # diagonal offset of first kt in group
                    if r0 >= 0:             # diagonal group: causal mask
                        nc.vector.tensor_mul(
                            e_sb[:], e_sb[:],
                            m4_sb[:, r0 * CHUNK : (r0 + 2) * CHUNK],
                        )
                    for i, kt in enumerate(kts):
                        nc.tensor.matmul(
                            ps_o[:],
                            v1[kt][:],
                            e_sb[:, i * CHUNK : (i + 1) * CHUNK],
                            start=(kt == 0), stop=(kt == nkt_j - 1),
                        )

                # ---- normalize: out[h, tq] * 1/denom[tq] ----
                rec_f = norm_pool.tile([H + 1, CHUNK], FP, tag="recf")
                nc.vector.reciprocal_approx_fast(rec_f[:], ps_o[:])
                rec = norm_pool.tile([H + 1, CHUNK], FPR, tag="rec")
                nc.vector.tensor_copy(rec[:], rec_f[:])
                ps_b = small_psum.tile([H, CHUNK], FP, tag="small")
                nc.tensor.matmul(
                    ps_b[:], ones_sb[H : H + 1, :], rec[H : H + 1, :],
                    start=True, stop=True,
                )
                bc_sb = norm_pool.tile([H, CHUNK], FP, tag="bc")
                nc.vector.tensor_copy(bc_sb[:], ps_b[:])
                nc.vector.tensor_mul(outT_sb[:, cs], ps_o[:H, :], bc_sb[:])
                nc.sync.dma_start(outT[:, cs], outT_sb[:, cs])

    nc.compile()
    return nc


def _marshal(a, Wk, Wq, Wv):
    aT = np.ascontiguousarray(a.transpose(0, 2, 1))            # [B, C, T]
    wqv = np.ascontiguousarray(
        np.concatenate([Wq * np.float32(SCALE), Wv], axis=1)
    )                                                          # [C, 128]
    idh = np.zeros((P, H), np.float32)
    idh[H:P, :] = np.eye(H, dtype=np.float32)
    m4 = np.zeros((P, 4 * CHUNK), np.float32)
    p = np.arange(P)[:, None]
    f = np.arange(CHUNK)[None, :]
    for r in range(4):
        m4[:, r * CHUNK : (r + 1) * CHUNK] = (f >= r * P + p).astype(np.float32)
    ones = np.ones((P, H), np.float32)
    return aT, wqv, np.ascontiguousarray(Wk), idh, m4, ones


def kernel(a, Wk, Wq, Wv):
    a = np.asarray(a, np.float32)
    Wk = np.asarray(Wk, np.float32)
    Wq = np.asarray(Wq, np.float32)
    Wv = np.asarray(Wv, np.float32)
    if "nc" not in _cache:
        _cache["nc"] = build_program()
    nc = _cache["nc"]

    aT, wqv, wk, idh, m4, ones = _marshal(a, Wk, Wq, Wv)
    in_maps = [
        {"aT": aT[b], "wqv": wqv, "wk": wk, "idh": idh, "m4": m4, "ones": ones}
        for b in range(B)
    ]
    res = bass_utils.run_bass_kernel_spmd(nc, in_maps, core_ids=list(range(B)))
    out = np.stack(
        [np.ascontiguousarray(res.results[b]["outT"].T) for b in range(B)]
    )
    return out.astype(np.float32)


# revision 10
# speedup vs baseline: 2.3767x; 1.0597x over previous
"""Single-head causal attention (B=8, T=2048, C=1024, H=64) on 8 TRN2 NeuronCores.

Data-parallel over batch: core b computes attention for batch element b.

Device algorithm (per core), all fp32:
  - Inputs pre-marshalled on host: aT = a.T  [C=1024, T=2048], Wqv = [Wq*scale | Wv]
    [1024, 128], Wk [1024, 64].
  - Projections: qT/vT from lhsT=Wqv tiles, kT from lhsT=Wk tiles, rhs = aT
    C-tiles; outputs land as qT [64, T] (partitions 0-63), vT (partitions
    64-127), kT [64, T].
  - v natural [T-tile, 65] built by PE transpose of vT 128-col chunks with an
    identity moving operand; column 64 is set to 1.0 (ones column).
  - Scores computed transposed: sT[tk, tq] via lhsT = kT tile [64, 128],
    rhs = qT chunk [64, 512] (contraction H=64).  exp on ScalarE directly from
    PSUM in [128, 1024] groups (2 k-tiles per op).  Causal mask = elementwise
    multiply with precomputed 0/1 masks on the 4 diagonal k-tiles per chunk.
  - PV: out_T/denom accumulate in one PSUM group: lhsT = [v | 1] [128, 65],
    rhs = expT group slices; row 64 of the [65, 512] accumulator is the
    softmax denominator (sum of exps).  No max-subtraction is needed: logits
    are ~N(0, ~1.5), max < ~10, exp is safely in fp32 range.
  - Normalize: reciprocal of denom row, broadcast across 64 partitions with a
    K=1 ones matmul, multiply, DMA out as outT [64, T].  Host transposes back.

T is processed in 4 chunks of 512 q-columns; aT is DMAed in T-quarters so
chunk j's entire dependency set (q, k, v cols <= 512(j+1)) arrives early and
compute overlaps the HBM stream.
"""

import sys

sys.path.insert(0, "/opt/trn_rl_repo")
sys.path.insert(0, "/root/.axon_site")

import numpy as np

import concourse.bass as bass
import concourse.mybir as mybir
import concourse.tile as tile
from concourse import bacc
from concourse import bass_utils

B, T, C, H = 8, 2048, 1024, 64
P = 128
NCT = C // P          # 8 C-tiles (contraction)
CHUNK = 512           # q-columns per chunk
NCH = T // CHUNK      # 4 chunks
NKT = T // P          # 16 k-tiles
SCALE = H ** -0.5
FP = mybir.dt.float32
FPR = mybir.dt.float32r   # 11-bit-mantissa RNE matmul mode, 3x faster than fp32

_cache = {}


def build_program():
    nc = bacc.Bacc("TRN2", target_bir_lowering=False, debug=False)

    aT = nc.dram_tensor("aT", [C, T], FPR, kind="ExternalInput").ap()
    wqv = nc.dram_tensor("wqv", [C, 2 * H], FPR, kind="ExternalInput").ap()
    wk = nc.dram_tensor("wk", [C, H], FPR, kind="ExternalInput").ap()
    idh = nc.dram_tensor("idh", [P, H], FPR, kind="ExternalInput").ap()
    m4 = nc.dram_tensor("m4", [P, 3 * P + CHUNK], FPR, kind="ExternalInput").ap()
    ones = nc.dram_tensor("ones", [P, H], FPR, kind="ExternalInput").ap()
    outT = nc.dram_tensor("outT", [H, T], FP, kind="ExternalOutput").ap()

    with tile.TileContext(nc) as tc:
        with (
            tc.tile_pool(name="const", bufs=1) as const_pool,
            tc.tile_pool(name="at", bufs=NCT * NCH) as at_pool,
            tc.tile_pool(name="qv", bufs=1) as qv_pool,
            tc.tile_pool(name="kt", bufs=1) as kt_pool,
            tc.tile_pool(name="v1", bufs=NKT) as v1_pool,
            tc.tile_pool(name="es", bufs=3) as e_pool,
            tc.tile_pool(name="norm", bufs=4) as norm_pool,
            tc.tile_pool(name="out", bufs=1) as out_pool,
            tc.tile_pool(name="ps_s", bufs=2, space="PSUM") as s_psum,
            tc.tile_pool(name="ps_proj", bufs=2, space="PSUM") as proj_psum,
            tc.tile_pool(name="ps_pv", bufs=1, space="PSUM") as pv_psum,
            tc.tile_pool(name="ps_small", bufs=1, space="PSUM") as small_psum,
        ):
            # ---- constants ----
            wqv_sb = const_pool.tile([P, NCT, 2 * H], FPR, tag="wqv")
            nc.gpsimd.dma_start(wqv_sb[:], wqv.rearrange("(ko p) m -> p ko m", p=P))
            wk_sb = const_pool.tile([P, NCT, H], FPR, tag="wk")
            nc.gpsimd.dma_start(wk_sb[:], wk.rearrange("(ko p) m -> p ko m", p=P))
            # ---- aT quarters, streamed in chunk order ----
            at_sb = {}
            for j in range(NCH):
                for c in range(NCT):
                    t_ = at_pool.tile([P, CHUNK], FPR, tag="at")
                    nc.sync.dma_start(
                        t_[:],
                        aT[c * P : (c + 1) * P, j * CHUNK : (j + 1) * CHUNK],
                    )
                    at_sb[(c, j)] = t_

            idh_sb = const_pool.tile([P, H], FPR, tag="idh")
            nc.gpsimd.dma_start(idh_sb[:], idh[:])
            m4_sb = const_pool.tile([P, 3 * P + CHUNK], FPR, tag="m4")
            nc.gpsimd.dma_start(m4_sb[:], m4[:])
            ones_sb = const_pool.tile([P, H], FPR, tag="ones")
            nc.gpsimd.dma_start(ones_sb[:], ones[:])

            qv_sb = qv_pool.tile([P, T], FPR, tag="qv")   # q rows 0-63, vT rows 64-127
            kT_sb = kt_pool.tile([H, T], FPR, tag="kt")
            outT_sb = out_pool.tile([H, T], FP, tag="ot")
            v1 = {}

            for j in range(NCH):
                cs = slice(j * CHUNK, (j + 1) * CHUNK)

                # ---- projections for this chunk of T ----
                ps_qv = proj_psum.tile([P, CHUNK], FP, tag="proj")
                for c in range(NCT):
                    nc.tensor.matmul(
                        ps_qv[:], wqv_sb[:, c, :], at_sb[(c, j)][:],
                        start=(c == 0), stop=(c == NCT - 1),
                    )
                ps_k = proj_psum.tile([P, CHUNK], FP, tag="proj")
                for c in range(NCT):
                    nc.tensor.matmul(
                        ps_k[:H], wk_sb[:, c, :], at_sb[(c, j)][:],
                        start=(c == 0), stop=(c == NCT - 1),
                    )
                nc.vector.tensor_copy(qv_sb[:, cs], ps_qv[:])
                nc.vector.tensor_copy(kT_sb[:, cs], ps_k[:H])

                # ---- v natural tiles ([v | 1], PE transpose of vT chunks) ----
                for kt in range(4 * j, 4 * j + 4):
                    ps_t = small_psum.tile([P, H], FPR, tag="small")
                    nc.tensor.transpose(
                        ps_t[:],
                        qv_sb[H:P, kt * P : (kt + 1) * P],
                        idh_sb[H:P, :],
                    )
                    vt = v1_pool.tile([P, H + 1], FPR, tag="v1")
                    nc.vector.tensor_copy(vt[:, H : H + 1], ones_sb[:, :1])
                    nc.vector.tensor_copy(vt[:, :H], ps_t[:])
                    v1[kt] = vt

                # ---- attention: groups of 2 k-tiles ----
                ps_o = pv_psum.tile([H + 1, CHUNK], FP, tag="pv")
                nkt_j = 4 * j + 4          # k-tiles for this chunk (causal)
                for g in range(nkt_j // 2):
                    kts = [2 * g, 2 * g + 1]
                    ps_s = s_psum.tile([P, 2 * CHUNK], FP, tag="s")
                    for i, kt in enumerate(kts):
                        nc.tensor.matmul(
                            ps_s[:, i * CHUNK : (i + 1) * CHUNK],
                            kT_sb[:, kt * P : (kt + 1) * P],
                            qv_sb[:H, cs],
                            start=True, stop=True,
                        )
                    e_sb = e_pool.tile([P, 2 * CHUNK], FPR, tag="e")
                    nc.scalar.activation(
                        e_sb[:], ps_s[:], mybir.ActivationFunctionType.Exp
                    )
                    r0 = 2 * g - 4 * j      # diagonal offset of first kt in group
                    if r0 >= 0:             # diagonal group: causal mask
                        for i, r in enumerate([r0, r0 + 1]):
                            nc.vector.tensor_mul(
                                e_sb[:, i * CHUNK : (i + 1) * CHUNK],
                                e_sb[:, i * CHUNK : (i + 1) * CHUNK],
                                m4_sb[:, 3 * P - P * r : 3 * P - P * r + CHUNK],
                            )
                    for i, kt in enumerate(kts):
                        nc.tensor.matmul(
                            ps_o[:],
                            v1[kt][:],
                            e_sb[:, i * CHUNK : (i + 1) * CHUNK],
                            start=(kt == 0), stop=(kt == nkt_j - 1),
                        )

                # ---- normalize: out[h, tq] * 1/denom[tq] ----
                rec_f = norm_pool.tile([H + 1, CHUNK], FP, tag="recf")
                nc.vector.reciprocal_approx_fast(rec_f[:], ps_o[:])
                rec = norm_pool.tile([H + 1, CHUNK], FPR, tag="rec")
                nc.vector.tensor_copy(rec[:], rec_f[:])
                ps_b = small_psum.tile([H, CHUNK], FP, tag="small")
                nc.tensor.matmul(
                    ps_b[:], ones_sb[H : H + 1, :], rec[H : H + 1, :],
                    start=True, stop=True,
                )
                bc_sb = norm_pool.tile([H, CHUNK], FP, tag="bc")
                nc.vector.tensor_copy(bc_sb[:], ps_b[:])
                nc.vector.tensor_mul(outT_sb[:, cs], ps_o[:H, :], bc_sb[:])
                nc.gpsimd.dma_start(outT[:, cs], outT_sb[:, cs])

    nc.compile()
    return nc


def _marshal(a, Wk, Wq, Wv):
    aT = np.ascontiguousarray(a.transpose(0, 2, 1))            # [B, C, T]
    wqv = np.ascontiguousarray(
        np.concatenate([Wq * np.float32(SCALE), Wv], axis=1)
    )                                                          # [C, 128]
    idh = np.zeros((P, H), np.float32)
    idh[H:P, :] = np.eye(H, dtype=np.float32)
    p = np.arange(P)[:, None]
    g = np.arange(3 * P + CHUNK)[None, :]
    m4 = (g >= p + 3 * P).astype(np.float32)
    ones = np.ones((P, H), np.float32)
    return aT, wqv, np.ascontiguousarray(Wk), idh, m4, ones


def kernel(a, Wk, Wq, Wv):
    a = np.asarray(a, np.float32)
    Wk = np.asarray(Wk, np.float32)
    Wq = np.asarray(Wq, np.float32)
    Wv = np.asarray(Wv, np.float32)
    if "nc" not in _cache:
        _cache["nc"] = build_program()
    nc = _cache["nc"]

    aT, wqv, wk, idh, m4, ones = _marshal(a, Wk, Wq, Wv)
    in_maps = [
        {"aT": aT[b], "wqv": wqv, "wk": wk, "idh": idh, "m4": m4, "ones": ones}
        for b in range(B)
    ]
    res = bass_utils.run_bass_kernel_spmd(nc, in_maps, core_ids=list(range(B)))
    out = np.stack(
        [np.ascontiguousarray(res.results[b]["outT"].T) for b in range(B)]
    )
    return out.astype(np.float32)
